# revision 1
# baseline (speedup 1.0000x reference)
"""Trainium2 Bass kernel for nn_AttentionDecoder (GRU decoder + dot attention).

Strategy (8 NeuronCores, data-parallel over batch, no collectives):
  - batch 64 -> 8 per core
  - Phase A (parallel): giT = W_ih @ embed^T for all timesteps (bf16 matmul)
  - Phase B (serial, 128 steps): GRU recurrence in transposed layout
    (gate-dim on partitions, batch on free dim). W_hh stationary bf16,
    h streamed as the 8-column moving operand. Gates are split into two
    h-chunks whose DVE/ACT chains are emitted as software-pipelined
    segments interleaved with the matmul stream (across step boundaries),
    so the in-order engine FIFOs rarely stall.
  - Phase C (parallel): attention per batch element via bf16 PE matmuls,
    free-dim softmax, PE transposes to assemble the output.

All matmuls use bf16 operands with f32 PSUM accumulation; gate arithmetic is
f32 (h is rounded to bf16 once per step). Host side does only sharding,
layout transposes, dtype casts, and the embedding gather.
"""

import numpy as np
import ml_dtypes

NB, S, H, E = 8, 128, 512, 512
G = 3 * H            # 1536
BT = NB * S          # 1024
NCORES = 8

_cache = {}


def _build():
    import concourse.bass as bass
    import concourse.bacc as bacc
    import concourse.mybir as mybir
    from concourse import tile
    from contextlib import ExitStack

    f32 = mybir.dt.float32
    bf16 = mybir.dt.bfloat16
    AF = mybir.ActivationFunctionType
    ALU = mybir.AluOpType
    PSUM = bass.MemorySpace.PSUM

    nc = bacc.Bacc(
        "TRN2",
        target_bir_lowering=False,
        debug=False,
        enable_asserts=False,
        num_devices=NCORES,
    )

    embedT_d = nc.dram_tensor("embedT", [E, BT], bf16, kind="ExternalInput")
    wih_d = nc.dram_tensor("W_ihT", [E, G], bf16, kind="ExternalInput")
    whh_d = nc.dram_tensor("W_hhT", [H, G], bf16, kind="ExternalInput")
    biascol_d = nc.dram_tensor("bias_col", [128, 12], f32, kind="ExternalInput")
    bhhn_d = nc.dram_tensor("bhh_n", [128, 4, NB], bf16, kind="ExternalInput")
    h0T_d = nc.dram_tensor("h0T", [H, NB], bf16, kind="ExternalInput")
    enc_d = nc.dram_tensor("enc", [NB, S, H], bf16, kind="ExternalInput")
    encT_d = nc.dram_tensor("encT", [NB, H, S], bf16, kind="ExternalInput")
    iden_d = nc.dram_tensor("iden", [128, 128], bf16, kind="ExternalInput")
    out_d = nc.dram_tensor("out", [NB, S, 2 * H], f32, kind="ExternalOutput")

    with tile.TileContext(nc) as tc, ExitStack() as ctx:
        cp = ctx.enter_context(tc.tile_pool(name="const", bufs=1))
        giT = cp.tile([128, 12, BT], f32)            # [p, g*4+hm, t*8+b]
        # h storage doubles as the recurrent state: column t holds h_{t-1}
        HallT = cp.tile([128, 4, NB, S + 1], bf16)   # [p, hm, b, t]
        whh = cp.tile([128, 4, G], bf16)
        wih = cp.tile([128, 4, G], bf16)
        embT = cp.tile([128, 4, BT], bf16)
        biascol = cp.tile([128, 12], f32)
        bhhn = cp.tile([128, 4, NB], bf16)
        iden = cp.tile([128, 128], bf16)

        nc.sync.dma_start(whh[:], whh_d.ap().rearrange("(k p) g -> p k g", p=128))
        nc.sync.dma_start(wih[:], wih_d.ap().rearrange("(k p) g -> p k g", p=128))
        nc.sync.dma_start(embT[:], embedT_d.ap().rearrange("(k p) n -> p k n", p=128))
        nc.sync.dma_start(biascol[:], biascol_d.ap())
        nc.sync.dma_start(bhhn[:], bhhn_d.ap())
        h0t = cp.tile([128, 4, NB], bf16)
        nc.sync.dma_start(h0t[:], h0T_d.ap().rearrange("(k p) b -> p k b", p=128))
        nc.vector.tensor_copy(HallT[:, :, :, 0], h0t[:])
        nc.sync.dma_start(iden[:], iden_d.ap())

        # ---- Phase A: giT[:, m, :] = (W_ih chunk) @ embedT + (b_ih [+ b_hh]) ----
        with tc.tile_pool(name="psA", bufs=4, space=PSUM) as psA:
            for m in range(12):
                for n in range(2):
                    psa = psA.tile([128, 512], f32, tag="psa")
                    for k in range(4):
                        nc.tensor.matmul(
                            psa[:],
                            wih[:, k, 128 * m : 128 * (m + 1)],
                            embT[:, k, 512 * n : 512 * (n + 1)],
                            start=(k == 0),
                            stop=(k == 3),
                        )
                    dst = giT[:, m, 512 * n : 512 * (n + 1)]
                    if (2 * m + n) % 2 == 0:
                        nc.vector.tensor_scalar_add(dst, psa[:], biascol[:, m : m + 1])
                    else:
                        nc.scalar.activation(
                            dst, psa[:], AF.Identity, bias=biascol[:, m : m + 1]
                        )

        # ---- Phase B: GRU recurrence, 128 serial steps ----
        # Weight m-index for gate g of h-chunk hm is m = 4*g + hm.
        # Two gate chunks (hm 0:2 and 2:4); for each, a 6-op DVE chain with
        # two ACT hops, software-pipelined so the in-order queues stay fed.
        # h lives in HallT column t (bf16); matmul rhs reads it strided.
        # PSUM groups are strictly sequential within the (single) bank:
        # for each m, its 4-5 matmuls (identity(b_hh) for n-gates, then
        # k0..k3) are consecutive. m-order is chunk-major so the first gate
        # chunk's inputs complete early; gate chains are software-pipelined
        # across the step boundary, tails on the GPSIMD engine.
        with (
            tc.tile_pool(name="psB", bufs=2, space=PSUM) as psB,
            tc.tile_pool(name="gp", bufs=3) as gp,
        ):
            state = {}   # (t, kind, c) -> tile, for cross-step pipelining

            def seg_a(st, psg, gig, c):  # c = 0 -> hm 0:2, c = 1 -> hm 2:4
                hs = slice(2 * c, 2 * c + 2)
                trz = gp.tile([128, 2, 2, NB], f32, tag=f"trz{c}", name=f"trz{c}")
                nc.vector.tensor_add(trz[:], psg[:, 0:2, hs, :], gig[:, 0:2, hs, :])
                rz = gp.tile([128, 2, 2, NB], f32, tag=f"rz{c}", name=f"rz{c}")
                nc.scalar.activation(rz[:], trz[:], AF.Sigmoid)
                state[(st, "rz", c)] = rz

            def seg_b(st, psg, gig, c):
                hs = slice(2 * c, 2 * c + 2)
                rz = state[(st, "rz", c)]
                tn2 = gp.tile([128, 2, NB], f32, tag=f"tn2{c}", name=f"tn2{c}")
                nc.vector.tensor_mul(tn2[:], rz[:, 0, :, :], psg[:, 2, hs, :])
                tn3 = gp.tile([128, 2, NB], f32, tag=f"tn3{c}", name=f"tn3{c}")
                nc.vector.tensor_add(tn3[:], tn2[:], gig[:, 2, hs, :])
                nn = gp.tile([128, 2, NB], f32, tag=f"nn{c}", name=f"nn{c}")
                nc.scalar.activation(nn[:], tn3[:], AF.Tanh)
                state[(st, "nn", c)] = nn

            def seg_c(st, c):
                hs = slice(2 * c, 2 * c + 2)
                rz = state.pop((st, "rz", c))
                nn = state.pop((st, "nn", c))
                th = gp.tile([128, 2, NB], f32, tag=f"th{c}", name=f"th{c}")
                nc.vector.tensor_sub(th[:], HallT[:, hs, :, st], nn[:])
                th2 = gp.tile([128, 2, NB], f32, tag=f"th2{c}", name=f"th2{c}")
                nc.vector.tensor_mul(th2[:], rz[:, 1, :, :], th[:])
                nc.vector.tensor_add(HallT[:, hs, :, st + 1], nn[:], th2[:])

            def m_group(psb, h_src, m, c):
                if m >= 8:   # open the n-gate group with the b_hh identity MM
                    nc.tensor.matmul(
                        psb[:, m, :], iden[:], bhhn[:, m - 8, :],
                        start=True, stop=False,
                    )
                for k in range(4):
                    nc.tensor.matmul(
                        psb[:, m, :],
                        whh[:, k, 128 * m : 128 * (m + 1)],
                        h_src[:, k, :],
                        start=(k == 0 and m < 8),
                        stop=(k == 3),
                    )

            prev = None  # (psg, gig) of step t-1 for cross-step tail segs
            for t in range(S):
                h_src = HallT[:, :, :, t]            # [128, 4, NB]
                psb = psB.tile([128, 12, NB], f32, tag="psb", name="psb")
                psg = psb[:].rearrange("p (g m) b -> p g m b", g=3)
                gig = giT[:, :, 8 * t : 8 * (t + 1)].rearrange(
                    "p (g m) b -> p g m b", g=3
                )
                if prev is not None:
                    seg_b(t - 1, *prev, 1)
                    seg_c(t - 1, 1)
                m_group(psb, h_src, 0, 0)
                m_group(psb, h_src, 1, 0)
                m_group(psb, h_src, 4, 0)
                m_group(psb, h_src, 5, 0)
                m_group(psb, h_src, 8, 0)
                m_group(psb, h_src, 9, 0)
                seg_a(t, psg, gig, 0)
                m_group(psb, h_src, 2, 1)
                m_group(psb, h_src, 3, 1)
                seg_b(t, psg, gig, 0)
                m_group(psb, h_src, 6, 1)
                seg_c(t, 0)
                m_group(psb, h_src, 7, 1)
                m_group(psb, h_src, 10, 1)
                m_group(psb, h_src, 11, 1)
                seg_a(t, psg, gig, 1)
                prev = (psg, gig)
            seg_b(S - 1, *prev, 1)
            seg_c(S - 1, 1)

        # ---- Phase C: attention + output assembly, per batch element ----
        with (
            tc.tile_pool(name="pc", bufs=2) as pc,
            tc.tile_pool(name="psC", bufs=2, space=PSUM) as psC,
            tc.tile_pool(name="psX", bufs=2, space=PSUM) as psX,
        ):
            for b in range(NB):
                encb = pc.tile([128, H], bf16, tag="encb")
                nc.sync.dma_start(encb[:], enc_d.ap()[b])
                enctb = pc.tile([128, 4, S], bf16, tag="enctb")
                nc.sync.dma_start(
                    enctb[:], encT_d.ap()[b].rearrange("(k p) s -> p k s", p=128)
                )
                ps_sc = psC.tile([128, 128], f32, tag="c128")
                for k in range(4):
                    nc.tensor.matmul(
                        ps_sc[:],
                        HallT[:, k, b, 1 : S + 1],
                        enctb[:, k, :],
                        start=(k == 0),
                        stop=(k == 3),
                    )
                mxn = pc.tile([128, 1], f32, tag="mxn")
                nc.vector.tensor_reduce(
                    mxn[:], ps_sc[:], op=ALU.max,
                    axis=mybir.AxisListType.X, negate=True,
                )
                probs = pc.tile([128, 128], bf16, tag="probs")
                sm = pc.tile([128, 1], f32, tag="sm")
                nc.scalar.activation(
                    probs[:], ps_sc[:], AF.Exp, bias=mxn[:], accum_out=sm[:]
                )
                rs = pc.tile([128, 1], f32, tag="rs")
                nc.vector.reciprocal(rs[:], sm[:])
                ps_pt = psC.tile([128, 128], bf16, tag="c128b", bufs=4)
                nc.tensor.transpose(ps_pt[:], probs[:], iden[:])
                probsT = pc.tile([128, 128], bf16, tag="probsT")
                nc.vector.tensor_copy(probsT[:], ps_pt[:])
                ps_cx = psX.tile([128, 512], f32, tag="ctx")
                nc.tensor.matmul(ps_cx[:], probsT[:], encb[:], start=True, stop=True)
                y = pc.tile([128, 2 * H], f32, tag="y")
                for k in range(4):
                    ps_h = psC.tile([128, 128], bf16, tag="c128b", bufs=4)
                    nc.tensor.transpose(ps_h[:], HallT[:, k, b, 1 : S + 1], iden[:])
                    nc.vector.tensor_copy(y[:, 128 * k : 128 * (k + 1)], ps_h[:])
                nc.vector.tensor_scalar_mul(y[:, H:], ps_cx[:], rs[:])
                nc.sync.dma_start(out_d.ap()[b], y[:])

    nc.compile()
    return nc


def _get_nc():
    if "nc" not in _cache:
        _cache["nc"] = _build()
    return _cache["nc"]


def prepare_in_maps(
    decoder_input,
    encoder_hidden,
    encoder_output,
    emb_table,
    W_ih,
    W_hh,
    b_ih,
    b_hh,
    epoch=0,
    **_unused,
):
    dec = np.asarray(decoder_input)
    enc_h = np.asarray(encoder_hidden, np.float32)[0]      # [64, 512]
    enc_o = np.asarray(encoder_output, np.float32)         # [64, 128, 512]
    emb = np.asarray(emb_table, np.float32)
    W_ih = np.asarray(W_ih, np.float32)
    W_hh = np.asarray(W_hh, np.float32)
    b_ih = np.asarray(b_ih, np.float32)
    b_hh = np.asarray(b_hh, np.float32)

    embed = emb[dec]                                       # [64, 128, 512] gather

    WihT_bf = np.ascontiguousarray(W_ih.T).astype(ml_dtypes.bfloat16)
    WhhT_bf = np.ascontiguousarray(W_hh.T).astype(ml_dtypes.bfloat16)
    # bias_col[:, m] = b_ih chunk m, plus b_hh chunk for r/z gates (m < 8)
    bias_col = np.zeros((128, 12), np.float32)
    for m in range(12):
        bias_col[:, m] = b_ih[128 * m : 128 * (m + 1)]
        if m < 8:
            bias_col[:, m] += b_hh[128 * m : 128 * (m + 1)]
    # bhh_n[p, k, b] = b_hh[1024 + 128k + p]
    bhh_n = np.ascontiguousarray(
        np.repeat(b_hh[1024:].reshape(4, 128).T[:, :, None], NB, axis=2)
    ).astype(ml_dtypes.bfloat16)
    iden = np.eye(128, dtype=ml_dtypes.bfloat16)

    in_maps = []
    for c in range(NCORES):
        bs = slice(c * NB, (c + 1) * NB)
        embedT = np.ascontiguousarray(
            embed[bs].transpose(2, 1, 0).reshape(E, BT)
        ).astype(ml_dtypes.bfloat16)                       # [E, t*8+b]
        enc_c = enc_o[bs]
        in_maps.append(
            {
                "embedT": embedT,
                "W_ihT": WihT_bf,
                "W_hhT": WhhT_bf,
                "bias_col": bias_col,
                "bhh_n": bhh_n,
                "h0T": np.ascontiguousarray(enc_h[bs].T).astype(ml_dtypes.bfloat16),
                "enc": np.ascontiguousarray(enc_c).astype(ml_dtypes.bfloat16),
                "encT": np.ascontiguousarray(
                    enc_c.transpose(0, 2, 1)
                ).astype(ml_dtypes.bfloat16),
                "iden": iden,
            }
        )
    return in_maps


def assemble(results):
    out = np.empty((NCORES * NB, S, 2 * H), np.float32)
    for c in range(NCORES):
        out[c * NB : (c + 1) * NB] = results[c]["out"]
    return out


def kernel(**inputs):
    from concourse.bass_utils import run_bass_kernel_spmd

    in_maps = prepare_in_maps(**inputs)
    nc = _get_nc()
    _cache["in_maps"] = in_maps
    res = run_bass_kernel_spmd(nc, in_maps, core_ids=list(range(NCORES)))
    return assemble(res.results)



# revision 3
# speedup vs baseline: 1.4805x; 1.4805x over previous
"""Trainium2 Bass kernel for nn_AttentionDecoder (GRU decoder + dot attention).

Strategy (8 NeuronCores, data-parallel over batch, no collectives):
  - batch 64 -> 8 per core
  - Phase A (parallel): gi = W_ih @ embed^T for all timesteps (bf16 matmul);
    r/z gate halves (+b_ih+b_hh) stored bf16, n-gate half (+b_ih) stored f32.
  - Phase B (serial, 128 steps): GRU recurrence in transposed layout
    (gate-dim on partitions, batch on free dim). The critical cycle is
    minimized: gi_rz and b_hh_n are injected into the PSUM accumulation via
    identity matmuls (PE is idle anyway), so the on-path chain per step is
    just  MM -> sigmoid(r) -> mul -> add -> tanh -> mul -> add .  The z-gate
    products ((1-z) via sigmoid(-x), z*h) are computed off the critical path.
    h-matmuls are ordered r-tiles, n-tiles, z-tiles so sigmoid(r) can start
    as early as possible.
  - Phase C (parallel): attention per batch element via bf16 PE matmuls,
    free-dim softmax, PE transposes to assemble the output. Encoder tiles are
    DMA-prefetched at kernel start; PSUM evacuations are split between the
    DVE and ACT engines.

All matmuls use bf16 operands with f32 PSUM accumulation; gate arithmetic is
f32 (h is rounded to bf16 once per step). Host side does only sharding,
layout transposes, dtype casts, and the embedding gather.
"""

import numpy as np
import ml_dtypes

NB, S, H, E = 8, 128, 512, 512
G = 3 * H            # 1536
BT = NB * S          # 1024
NCORES = 8

_cache = {}


def _build():
    import concourse.bass as bass
    import concourse.bacc as bacc
    import concourse.mybir as mybir
    from concourse import tile
    from contextlib import ExitStack

    f32 = mybir.dt.float32
    bf16 = mybir.dt.bfloat16
    AF = mybir.ActivationFunctionType
    ALU = mybir.AluOpType
    PSUM = bass.MemorySpace.PSUM

    nc = bacc.Bacc(
        "TRN2",
        target_bir_lowering=False,
        debug=False,
        enable_asserts=False,
        num_devices=NCORES,
    )

    embedT_d = nc.dram_tensor("embedT", [E, BT], bf16, kind="ExternalInput")
    wih_d = nc.dram_tensor("W_ihT", [E, G], bf16, kind="ExternalInput")
    whh_d = nc.dram_tensor("W_hhT", [H, G], bf16, kind="ExternalInput")
    biascol_d = nc.dram_tensor("bias_col", [128, 12], f32, kind="ExternalInput")
    bhhn_d = nc.dram_tensor("bhh_n", [128, 4, NB], bf16, kind="ExternalInput")
    h0T_d = nc.dram_tensor("h0T", [H, NB], bf16, kind="ExternalInput")
    enc_d = nc.dram_tensor("enc", [NB, S, H], bf16, kind="ExternalInput")
    encT_d = nc.dram_tensor("encT", [NB, H, S], bf16, kind="ExternalInput")
    iden_d = nc.dram_tensor("iden", [128, 128], bf16, kind="ExternalInput")
    out_d = nc.dram_tensor("out", [NB, S, 2 * H], f32, kind="ExternalOutput")

    with tile.TileContext(nc) as tc, ExitStack() as ctx:
        cp = ctx.enter_context(tc.tile_pool(name="const", bufs=1))
        giRZ = cp.tile([128, 8, BT], bf16)           # r/z gate inputs (+biases)
        giN = cp.tile([128, 4, BT], f32)             # n gate inputs (+b_ih)
        # h storage doubles as the recurrent state: column t holds h_{t-1}
        HallT = cp.tile([128, 4, NB, S + 1], bf16)   # [p, hm, b, t]
        whh = cp.tile([128, 4, G], bf16)
        wih = cp.tile([128, 4, G], bf16)
        embT = cp.tile([128, 4, BT], bf16)
        biascol = cp.tile([128, 12], f32)
        bhhn = cp.tile([128, 4, NB], bf16)
        iden = cp.tile([128, 128], bf16)
        encAll = cp.tile([128, NB, H], bf16)         # enc[b]: [s, h]
        encTAll = cp.tile([128, NB, 4, S], bf16)     # encT[b]: [p, hm, s]

        nc.sync.dma_start(whh[:], whh_d.ap().rearrange("(k p) g -> p k g", p=128))
        nc.sync.dma_start(wih[:], wih_d.ap().rearrange("(k p) g -> p k g", p=128))
        nc.sync.dma_start(embT[:], embedT_d.ap().rearrange("(k p) n -> p k n", p=128))
        nc.sync.dma_start(biascol[:], biascol_d.ap())
        nc.sync.dma_start(bhhn[:], bhhn_d.ap())
        h0t = cp.tile([128, 4, NB], bf16)
        nc.sync.dma_start(h0t[:], h0T_d.ap().rearrange("(k p) b -> p k b", p=128))
        nc.vector.tensor_copy(HallT[:, :, :, 0], h0t[:])
        nc.sync.dma_start(iden[:], iden_d.ap())
        for b in range(NB):
            nc.sync.dma_start(encAll[:, b, :], enc_d.ap()[b])
            nc.sync.dma_start(
                encTAll[:, b], encT_d.ap()[b].rearrange("(k p) s -> p k s", p=128)
            )

        # ---- Phase A: gi[:, m, :] = (W_ih chunk) @ embedT + biases ----
        with tc.tile_pool(name="psA", bufs=4, space=PSUM) as psA:
            for m in range(12):
                for n in range(2):
                    psa = psA.tile([128, 512], f32, tag="psa")
                    for k in range(4):
                        nc.tensor.matmul(
                            psa[:],
                            wih[:, k, 128 * m : 128 * (m + 1)],
                            embT[:, k, 512 * n : 512 * (n + 1)],
                            start=(k == 0),
                            stop=(k == 3),
                        )
                    if m < 8:
                        dst = giRZ[:, m, 512 * n : 512 * (n + 1)]
                    else:
                        dst = giN[:, m - 8, 512 * n : 512 * (n + 1)]
                    if (2 * m + n) % 2 == 0:
                        nc.vector.tensor_scalar_add(dst, psa[:], biascol[:, m : m + 1])
                    else:
                        nc.scalar.activation(
                            dst, psa[:], AF.Identity, bias=biascol[:, m : m + 1]
                        )

        # ---- Phase B: GRU recurrence, 128 serial steps ----
        # Weight m-index: m 0..3 = r gates, 4..7 = z gates, 8..11 = n gates
        # (unit chunk = m % 4 in each gate). psb[:, m, b] accumulates
        # identity-MM(gi_rz or b_hh_n) + sum_k W_hh[m,k] @ h[k].
        # Critical path: h-MMs(r) -> sigmoid(r) -> tn2 -> tn3 -> tanh ->
        # pp -> hnew; everything else (sigmoid(-z), z*h products) is
        # off-path.
        MORDER = [0, 1, 2, 3, 8, 9, 10, 11, 4, 5, 6, 7]
        with (
            tc.tile_pool(name="psB", bufs=2, space=PSUM) as psB,
            tc.tile_pool(name="gp", bufs=3) as gp,
        ):
            for t in range(S):
                h_src = HallT[:, :, :, t]            # [128, 4, NB]
                gsl = slice(8 * t, 8 * (t + 1))
                psb = psB.tile([128, 12, NB], f32, tag="psb", name="psb")
                # off-path: open each m-group's accumulation with an
                # identity matmul injecting gi_rz (m<8) or b_hh_n (m>=8).
                # Only the first id-MM clears the bank's has_written bits
                # (start=True wipes the WHOLE bank); the others then
                # overwrite their stale slices and set their own bits, and
                # the h-matmuls accumulate on top.
                for m in range(8):
                    nc.tensor.matmul(
                        psb[:, m, :], iden[:], giRZ[:, m, gsl],
                        start=(m == 0), stop=False,
                    )
                for m in range(8, 12):
                    nc.tensor.matmul(
                        psb[:, m, :], iden[:], bhhn[:, m - 8, :],
                        start=False, stop=False,
                    )
                # h-dependent matmuls: r tiles, n tiles, z tiles
                for m in MORDER:
                    for k in range(4):
                        nc.tensor.matmul(
                            psb[:, m, :],
                            whh[:, k, 128 * m : 128 * (m + 1)],
                            h_src[:, k, :],
                            start=False,
                            stop=(k == 3),
                        )
                rr = gp.tile([128, 4, NB], f32, tag="rr", name="rr")
                nc.scalar.activation(rr[:], psb[:, 0:4, :], AF.Sigmoid)
                uu = gp.tile([128, 4, NB], f32, tag="uu", name="uu")  # 1 - z
                nc.scalar.activation(uu[:], psb[:, 4:8, :], AF.Sigmoid, scale=-1.0)
                tn2 = gp.tile([128, 4, NB], f32, tag="tn2", name="tn2")
                nc.vector.tensor_mul(tn2[:], rr[:], psb[:, 8:12, :])
                tn3 = gp.tile([128, 4, NB], f32, tag="tn3", name="tn3")
                nc.vector.tensor_add(tn3[:], tn2[:], giN[:, :, gsl])
                nn = gp.tile([128, 4, NB], f32, tag="nn", name="nn")
                nc.scalar.activation(nn[:], tn3[:], AF.Tanh)
                # off-path: v = z*h = h - (1-z)*h
                qq = gp.tile([128, 4, NB], f32, tag="qq", name="qq")
                nc.vector.tensor_mul(qq[:], uu[:], h_src)
                vv = gp.tile([128, 4, NB], f32, tag="vv", name="vv")
                nc.vector.tensor_sub(vv[:], h_src, qq[:])
                # on-path tail: hnew = (1-z)*n + z*h
                pp = gp.tile([128, 4, NB], f32, tag="pp", name="pp")
                nc.vector.tensor_mul(pp[:], uu[:], nn[:])
                nc.vector.tensor_add(HallT[:, :, :, t + 1], pp[:], vv[:])

        # ---- Phase C: attention + output assembly, per batch element ----
        with (
            tc.tile_pool(name="pc", bufs=2) as pc,
            tc.tile_pool(name="psC", bufs=2, space=PSUM) as psC,
            tc.tile_pool(name="psX", bufs=2, space=PSUM) as psX,
        ):
            for b in range(NB):
                ps_sc = psC.tile([128, 128], f32, tag="c128")
                for k in range(4):
                    nc.tensor.matmul(
                        ps_sc[:],
                        HallT[:, k, b, 1 : S + 1],
                        encTAll[:, b, k, :],
                        start=(k == 0),
                        stop=(k == 3),
                    )
                mxn = pc.tile([128, 1], f32, tag="mxn")
                nc.vector.tensor_reduce(
                    mxn[:], ps_sc[:], op=ALU.max,
                    axis=mybir.AxisListType.X, negate=True,
                )
                probs = pc.tile([128, 128], bf16, tag="probs")
                sm = pc.tile([128, 1], f32, tag="sm")
                nc.scalar.activation(
                    probs[:], ps_sc[:], AF.Exp, bias=mxn[:], accum_out=sm[:]
                )
                rs = pc.tile([128, 1], f32, tag="rs")
                nc.vector.reciprocal(rs[:], sm[:])
                ps_pt = psC.tile([128, 128], bf16, tag="c128b", bufs=4)
                nc.tensor.transpose(ps_pt[:], probs[:], iden[:])
                probsT = pc.tile([128, 128], bf16, tag="probsT")
                nc.scalar.activation(probsT[:], ps_pt[:], AF.Copy)
                ps_cx = psX.tile([128, 512], f32, tag="ctx")
                nc.tensor.matmul(
                    ps_cx[:], probsT[:], encAll[:, b, :], start=True, stop=True
                )
                y = pc.tile([128, 2 * H], f32, tag="y")
                for k in range(4):
                    ps_h = psC.tile([128, 128], bf16, tag="c128b", bufs=4)
                    nc.tensor.transpose(ps_h[:], HallT[:, k, b, 1 : S + 1], iden[:])
                    if k % 2 == 0:
                        nc.vector.tensor_copy(y[:, 128 * k : 128 * (k + 1)], ps_h[:])
                    else:
                        nc.scalar.activation(
                            y[:, 128 * k : 128 * (k + 1)], ps_h[:], AF.Copy
                        )
                nc.vector.tensor_scalar_mul(y[:, H:], ps_cx[:], rs[:])
                nc.sync.dma_start(out_d.ap()[b], y[:])

    nc.compile()
    return nc


def _get_nc():
    if "nc" not in _cache:
        _cache["nc"] = _build()
    return _cache["nc"]


def prepare_in_maps(
    decoder_input,
    encoder_hidden,
    encoder_output,
    emb_table,
    W_ih,
    W_hh,
    b_ih,
    b_hh,
    epoch=0,
    **_unused,
):
    dec = np.asarray(decoder_input)
    enc_h = np.asarray(encoder_hidden, np.float32)[0]      # [64, 512]
    enc_o = np.asarray(encoder_output, np.float32)         # [64, 128, 512]
    emb = np.asarray(emb_table, np.float32)
    W_ih = np.asarray(W_ih, np.float32)
    W_hh = np.asarray(W_hh, np.float32)
    b_ih = np.asarray(b_ih, np.float32)
    b_hh = np.asarray(b_hh, np.float32)

    embed = emb[dec]                                       # [64, 128, 512] gather

    WihT_bf = np.ascontiguousarray(W_ih.T).astype(ml_dtypes.bfloat16)
    WhhT_bf = np.ascontiguousarray(W_hh.T).astype(ml_dtypes.bfloat16)
    # bias_col[:, m] = b_ih chunk m, plus b_hh chunk for r/z gates (m < 8)
    bias_col = np.zeros((128, 12), np.float32)
    for m in range(12):
        bias_col[:, m] = b_ih[128 * m : 128 * (m + 1)]
        if m < 8:
            bias_col[:, m] += b_hh[128 * m : 128 * (m + 1)]
    # bhh_n[p, k, b] = b_hh[1024 + 128k + p]
    bhh_n = np.ascontiguousarray(
        np.repeat(b_hh[1024:].reshape(4, 128).T[:, :, None], NB, axis=2)
    ).astype(ml_dtypes.bfloat16)
    iden = np.eye(128, dtype=ml_dtypes.bfloat16)

    in_maps = []
    for c in range(NCORES):
        bs = slice(c * NB, (c + 1) * NB)
        embedT = np.ascontiguousarray(
            embed[bs].transpose(2, 1, 0).reshape(E, BT)
        ).astype(ml_dtypes.bfloat16)                       # [E, t*8+b]
        enc_c = enc_o[bs]
        in_maps.append(
            {
                "embedT": embedT,
                "W_ihT": WihT_bf,
                "W_hhT": WhhT_bf,
                "bias_col": bias_col,
                "bhh_n": bhh_n,
                "h0T": np.ascontiguousarray(enc_h[bs].T).astype(ml_dtypes.bfloat16),
                "enc": np.ascontiguousarray(enc_c).astype(ml_dtypes.bfloat16),
                "encT": np.ascontiguousarray(
                    enc_c.transpose(0, 2, 1)
                ).astype(ml_dtypes.bfloat16),
                "iden": iden,
            }
        )
    return in_maps


def assemble(results):
    out = np.empty((NCORES * NB, S, 2 * H), np.float32)
    for c in range(NCORES):
        out[c * NB : (c + 1) * NB] = results[c]["out"]
    return out


def kernel(**inputs):
    from concourse.bass_utils import run_bass_kernel_spmd

    in_maps = prepare_in_maps(**inputs)
    nc = _get_nc()
    _cache["in_maps"] = in_maps
    res = run_bass_kernel_spmd(nc, in_maps, core_ids=list(range(NCORES)))
    return assemble(res.results)


# revision 5
# speedup vs baseline: 1.5912x; 1.0748x over previous
"""Trainium2 Bass kernel for nn_AttentionDecoder (GRU decoder + dot attention).

Strategy (8 NeuronCores, data-parallel over batch, no collectives):
  - batch 64 -> 8 per core
  - Phase A (parallel): gi = W_ih @ embed^T for all timesteps (bf16 matmul);
    r/z gate halves (+b_ih+b_hh) stored bf16, n-gate half (+b_ih) stored f32.
  - Phase B (serial, 128 steps): GRU recurrence in transposed layout
    (gate-dim on partitions, batch on free dim). The critical cycle is
    minimized: gi_rz and b_hh_n are injected into the PSUM accumulation via
    identity matmuls (PE is idle anyway), so the on-path chain per step is
    just  MM -> sigmoid(r) -> mul -> add -> tanh -> mul -> add .  The z-gate
    products ((1-z) via sigmoid(-x), z*h) are computed off the critical path.
    h-matmuls are ordered r-tiles, n-tiles, z-tiles so sigmoid(r) can start
    as early as possible.
  - Phase C (parallel): attention per batch element via bf16 PE matmuls,
    free-dim softmax, PE transposes to assemble the output. Encoder tiles are
    DMA-prefetched at kernel start; PSUM evacuations are split between the
    DVE and ACT engines.

All matmuls use bf16 operands with f32 PSUM accumulation; gate arithmetic is
f32 (h is rounded to bf16 once per step). Host side does only sharding,
layout transposes, dtype casts, and the embedding gather.
"""

import numpy as np
import ml_dtypes

NB, S, H, E = 8, 128, 512, 512
G = 3 * H            # 1536
BT = NB * S          # 1024
NCORES = 8

_cache = {}


def _build():
    import concourse.bass as bass
    import concourse.bacc as bacc
    import concourse.mybir as mybir
    from concourse import tile
    from contextlib import ExitStack

    f32 = mybir.dt.float32
    bf16 = mybir.dt.bfloat16
    AF = mybir.ActivationFunctionType
    ALU = mybir.AluOpType
    PSUM = bass.MemorySpace.PSUM

    nc = bacc.Bacc(
        "TRN2",
        target_bir_lowering=False,
        debug=False,
        enable_asserts=False,
        num_devices=NCORES,
    )

    embedT_d = nc.dram_tensor("embedT", [E, BT], bf16, kind="ExternalInput")
    wih_d = nc.dram_tensor("W_ihT", [E, G], bf16, kind="ExternalInput")
    whh_d = nc.dram_tensor("W_hhT", [H, G], bf16, kind="ExternalInput")
    biascol_d = nc.dram_tensor("bias_col", [128, 12], f32, kind="ExternalInput")
    bhhn_d = nc.dram_tensor("bhh_n", [128, 4, NB], bf16, kind="ExternalInput")
    h0T_d = nc.dram_tensor("h0T", [H, NB], bf16, kind="ExternalInput")
    enc_d = nc.dram_tensor("enc", [NB, S, H], bf16, kind="ExternalInput")
    encT_d = nc.dram_tensor("encT", [NB, H, S], bf16, kind="ExternalInput")
    iden_d = nc.dram_tensor("iden", [128, 128], bf16, kind="ExternalInput")
    out_d = nc.dram_tensor("out", [NB, S, 2 * H], f32, kind="ExternalOutput")

    with tile.TileContext(nc) as tc, ExitStack() as ctx:
        cp = ctx.enter_context(tc.tile_pool(name="const", bufs=1))
        giRZ = cp.tile([128, 8, BT], bf16)           # r/z gate inputs (+biases)
        giN = cp.tile([128, 4, BT], f32)             # n gate inputs (+b_ih)
        # h storage doubles as the recurrent state: column t holds h_{t-1}
        HallT = cp.tile([128, 4, NB, S + 1], bf16)   # [p, hm, b, t]
        whh = cp.tile([128, 4, G], bf16)
        wih = cp.tile([128, 4, G], bf16)
        embT = cp.tile([128, 4, BT], bf16)
        biascol = cp.tile([128, 12], f32)
        bhhn = cp.tile([128, 4, NB], bf16)
        iden = cp.tile([128, 128], bf16)
        encAll = cp.tile([128, NB, H], bf16)         # enc[b]: [s, h]
        encTAll = cp.tile([128, NB, 4, S], bf16)     # encT[b]: [p, hm, s]

        nc.sync.dma_start(whh[:], whh_d.ap().rearrange("(k p) g -> p k g", p=128))
        nc.sync.dma_start(wih[:], wih_d.ap().rearrange("(k p) g -> p k g", p=128))
        nc.sync.dma_start(embT[:], embedT_d.ap().rearrange("(k p) n -> p k n", p=128))
        nc.sync.dma_start(biascol[:], biascol_d.ap())
        nc.sync.dma_start(bhhn[:], bhhn_d.ap())
        h0t = cp.tile([128, 4, NB], bf16)
        nc.sync.dma_start(h0t[:], h0T_d.ap().rearrange("(k p) b -> p k b", p=128))
        nc.vector.tensor_copy(HallT[:, :, :, 0], h0t[:])
        nc.sync.dma_start(iden[:], iden_d.ap())
        for b in range(NB):
            nc.sync.dma_start(encAll[:, b, :], enc_d.ap()[b])
            nc.sync.dma_start(
                encTAll[:, b], encT_d.ap()[b].rearrange("(k p) s -> p k s", p=128)
            )

        # ---- Phase A: gi[:, m, :] = (W_ih chunk) @ embedT + biases ----
        with tc.tile_pool(name="psA", bufs=4, space=PSUM) as psA:
            for m in range(12):
                for n in range(2):
                    psa = psA.tile([128, 512], f32, tag="psa")
                    for k in range(4):
                        nc.tensor.matmul(
                            psa[:],
                            wih[:, k, 128 * m : 128 * (m + 1)],
                            embT[:, k, 512 * n : 512 * (n + 1)],
                            start=(k == 0),
                            stop=(k == 3),
                        )
                    if m < 8:
                        dst = giRZ[:, m, 512 * n : 512 * (n + 1)]
                    else:
                        dst = giN[:, m - 8, 512 * n : 512 * (n + 1)]
                    if (2 * m + n) % 2 == 0:
                        nc.vector.tensor_scalar_add(dst, psa[:], biascol[:, m : m + 1])
                    else:
                        nc.scalar.activation(
                            dst, psa[:], AF.Identity, bias=biascol[:, m : m + 1]
                        )

        # ---- Phase B: GRU recurrence, 128 serial steps ----
        # Weight m-index: m 0..3 = r gates, 4..7 = z gates (host-negated so
        # sigmoid gives 1-z directly), 8..11 = n gates. The r/z gates and the
        # n gate accumulate in SEPARATE PSUM banks so the sigmoid (which
        # Tile gates on whole-bank PE-write completion) does not wait for the
        # n matmuls. Critical path per step:
        #   h-MMs(rz) -> sigmoid(r|1-z) -> tn2 -> tn3 -> tanh -> pp -> hnew
        # The z*h products run off-path on the idle GPSIMD engine.
        with (
            tc.tile_pool(name="psB", bufs=2, space=PSUM) as psB,
            tc.tile_pool(name="gp", bufs=3) as gp,
        ):
            for t in range(S):
                h_src = HallT[:, :, :, t]            # [128, 4, NB]
                gsl = slice(8 * t, 8 * (t + 1))
                psrz = psB.tile([128, 8, NB], f32, tag="psrz", name="psrz")
                psn = psB.tile([128, 4, NB], f32, tag="psn", name="psn")
                # off-path: open the accumulations with identity matmuls
                # injecting gi_rz / b_hh_n. Only the first id-MM per bank
                # clears has_written (start=True wipes the WHOLE bank); the
                # others overwrite their stale slices, and the h-matmuls
                # accumulate on top.
                for m in range(8):
                    nc.tensor.matmul(
                        psrz[:, m, :], iden[:], giRZ[:, m, gsl],
                        start=(m == 0), stop=False,
                    )
                for j in range(4):
                    nc.tensor.matmul(
                        psn[:, j, :], iden[:], bhhn[:, j, :],
                        start=(j == 0), stop=False,
                    )
                # h-dependent matmuls: rz tiles first, then n tiles
                for m in range(8):
                    for k in range(4):
                        nc.tensor.matmul(
                            psrz[:, m, :],
                            whh[:, k, 128 * m : 128 * (m + 1)],
                            h_src[:, k, :],
                            start=False,
                            stop=(k == 3),
                        )
                for m in range(8, 12):
                    for k in range(4):
                        nc.tensor.matmul(
                            psn[:, m - 8, :],
                            whh[:, k, 128 * m : 128 * (m + 1)],
                            h_src[:, k, :],
                            start=False,
                            stop=(k == 3),
                        )
                ru = gp.tile([128, 8, NB], f32, tag="ru", name="ru")
                nc.scalar.activation(ru[:], psrz[:], AF.Sigmoid)
                tn2 = gp.tile([128, 4, NB], f32, tag="tn2", name="tn2")
                nc.vector.tensor_mul(tn2[:], ru[:, 0:4, :], psn[:])
                tn3 = gp.tile([128, 4, NB], f32, tag="tn3", name="tn3")
                nc.vector.tensor_add(tn3[:], tn2[:], giN[:, :, gsl])
                nn = gp.tile([128, 4, NB], f32, tag="nn", name="nn")
                nc.scalar.activation(nn[:], tn3[:], AF.Tanh)
                # off-path on GPSIMD: v = z*h = h - (1-z)*h
                qq = gp.tile([128, 4, NB], f32, tag="qq", name="qq")
                nc.gpsimd.tensor_mul(qq[:], ru[:, 4:8, :], h_src)
                vv = gp.tile([128, 4, NB], f32, tag="vv", name="vv")
                nc.gpsimd.tensor_sub(vv[:], h_src, qq[:])
                # on-path tail: hnew = (1-z)*n + z*h
                pp = gp.tile([128, 4, NB], f32, tag="pp", name="pp")
                nc.vector.tensor_mul(pp[:], ru[:, 4:8, :], nn[:])
                nc.vector.tensor_add(HallT[:, :, :, t + 1], pp[:], vv[:])

        # ---- Phase C: attention + output assembly, per batch element ----
        with (
            tc.tile_pool(name="pc", bufs=2) as pc,
            tc.tile_pool(name="psC", bufs=2, space=PSUM) as psC,
            tc.tile_pool(name="psX", bufs=2, space=PSUM) as psX,
        ):
            for b in range(NB):
                ps_sc = psC.tile([128, 128], f32, tag="c128")
                for k in range(4):
                    nc.tensor.matmul(
                        ps_sc[:],
                        HallT[:, k, b, 1 : S + 1],
                        encTAll[:, b, k, :],
                        start=(k == 0),
                        stop=(k == 3),
                    )
                mxn = pc.tile([128, 1], f32, tag="mxn")
                nc.vector.tensor_reduce(
                    mxn[:], ps_sc[:], op=ALU.max,
                    axis=mybir.AxisListType.X, negate=True,
                )
                probs = pc.tile([128, 128], bf16, tag="probs")
                sm = pc.tile([128, 1], f32, tag="sm")
                nc.scalar.activation(
                    probs[:], ps_sc[:], AF.Exp, bias=mxn[:], accum_out=sm[:]
                )
                rs = pc.tile([128, 1], f32, tag="rs")
                nc.vector.reciprocal(rs[:], sm[:])
                ps_pt = psC.tile([128, 128], bf16, tag="c128b", bufs=4)
                nc.tensor.transpose(ps_pt[:], probs[:], iden[:])
                probsT = pc.tile([128, 128], bf16, tag="probsT")
                nc.scalar.activation(probsT[:], ps_pt[:], AF.Copy)
                ps_cx = psX.tile([128, 512], f32, tag="ctx")
                nc.tensor.matmul(
                    ps_cx[:], probsT[:], encAll[:, b, :], start=True, stop=True
                )
                y = pc.tile([128, 2 * H], f32, tag="y")
                for k in range(4):
                    ps_h = psC.tile([128, 128], bf16, tag="c128b", bufs=4)
                    nc.tensor.transpose(ps_h[:], HallT[:, k, b, 1 : S + 1], iden[:])
                    if k % 2 == 0:
                        nc.vector.tensor_copy(y[:, 128 * k : 128 * (k + 1)], ps_h[:])
                    else:
                        nc.scalar.activation(
                            y[:, 128 * k : 128 * (k + 1)], ps_h[:], AF.Copy
                        )
                nc.vector.tensor_scalar_mul(y[:, H:], ps_cx[:], rs[:])
                nc.sync.dma_start(out_d.ap()[b], y[:])

    nc.compile()
    return nc


def _get_nc():
    if "nc" not in _cache:
        _cache["nc"] = _build()
    return _cache["nc"]


def prepare_in_maps(
    decoder_input,
    encoder_hidden,
    encoder_output,
    emb_table,
    W_ih,
    W_hh,
    b_ih,
    b_hh,
    epoch=0,
    **_unused,
):
    dec = np.asarray(decoder_input)
    enc_h = np.asarray(encoder_hidden, np.float32)[0]      # [64, 512]
    enc_o = np.asarray(encoder_output, np.float32)         # [64, 128, 512]
    emb = np.asarray(emb_table, np.float32)
    W_ih = np.asarray(W_ih, np.float32)
    W_hh = np.asarray(W_hh, np.float32)
    b_ih = np.asarray(b_ih, np.float32)
    b_hh = np.asarray(b_hh, np.float32)

    embed = emb[dec]                                       # [64, 128, 512] gather

    # Negate the z-gate rows (512:1024) of weights and biases so the device
    # computes -x_z in PSUM and a single sigmoid yields [r | 1-z] directly.
    W_ih = W_ih.copy(); W_ih[512:1024] *= -1.0
    W_hh = W_hh.copy(); W_hh[512:1024] *= -1.0
    b_ih = b_ih.copy(); b_ih[512:1024] *= -1.0
    b_hh = b_hh.copy(); b_hh[512:1024] *= -1.0

    WihT_bf = np.ascontiguousarray(W_ih.T).astype(ml_dtypes.bfloat16)
    WhhT_bf = np.ascontiguousarray(W_hh.T).astype(ml_dtypes.bfloat16)
    # bias_col[:, m] = b_ih chunk m, plus b_hh chunk for r/z gates (m < 8)
    bias_col = np.zeros((128, 12), np.float32)
    for m in range(12):
        bias_col[:, m] = b_ih[128 * m : 128 * (m + 1)]
        if m < 8:
            bias_col[:, m] += b_hh[128 * m : 128 * (m + 1)]
    # bhh_n[p, k, b] = b_hh[1024 + 128k + p]
    bhh_n = np.ascontiguousarray(
        np.repeat(b_hh[1024:].reshape(4, 128).T[:, :, None], NB, axis=2)
    ).astype(ml_dtypes.bfloat16)
    iden = np.eye(128, dtype=ml_dtypes.bfloat16)

    in_maps = []
    for c in range(NCORES):
        bs = slice(c * NB, (c + 1) * NB)
        embedT = np.ascontiguousarray(
            embed[bs].transpose(2, 1, 0).reshape(E, BT)
        ).astype(ml_dtypes.bfloat16)                       # [E, t*8+b]
        enc_c = enc_o[bs]
        in_maps.append(
            {
                "embedT": embedT,
                "W_ihT": WihT_bf,
                "W_hhT": WhhT_bf,
                "bias_col": bias_col,
                "bhh_n": bhh_n,
                "h0T": np.ascontiguousarray(enc_h[bs].T).astype(ml_dtypes.bfloat16),
                "enc": np.ascontiguousarray(enc_c).astype(ml_dtypes.bfloat16),
                "encT": np.ascontiguousarray(
                    enc_c.transpose(0, 2, 1)
                ).astype(ml_dtypes.bfloat16),
                "iden": iden,
            }
        )
    return in_maps


def assemble(results):
    out = np.empty((NCORES * NB, S, 2 * H), np.float32)
    for c in range(NCORES):
        out[c * NB : (c + 1) * NB] = results[c]["out"]
    return out


def kernel(**inputs):
    from concourse.bass_utils import run_bass_kernel_spmd

    in_maps = prepare_in_maps(**inputs)
    nc = _get_nc()
    _cache["in_maps"] = in_maps
    res = run_bass_kernel_spmd(nc, in_maps, core_ids=list(range(NCORES)))
    return assemble(res.results)


# revision 9
# speedup vs baseline: 2.1289x; 1.3379x over previous
"""Trainium2 Bass kernel for nn_AttentionDecoder (GRU decoder + dot attention).

Strategy (8 NeuronCores, data-parallel over batch, no collectives):
  - batch 64 -> 8 per core
  - Phase A (parallel): gi = W_ih @ embed^T for all timesteps (bf16 matmul);
    r/z gate halves (+b_ih+b_hh) stored bf16, n-gate half (+b_ih) stored f32.
  - Phase B (serial, 128 steps): GRU recurrence in transposed layout
    (gate-dim on partitions, batch on free dim). The critical cycle is
    minimized: gi_rz and b_hh_n are injected into the PSUM accumulation via
    identity matmuls (PE is idle anyway), so the on-path chain per step is
    just  MM -> sigmoid(r) -> mul -> add -> tanh -> mul -> add .  The z-gate
    products ((1-z) via sigmoid(-x), z*h) are computed off the critical path.
    h-matmuls are ordered r-tiles, n-tiles, z-tiles so sigmoid(r) can start
    as early as possible.
  - Phase C (parallel): attention per batch element via bf16 PE matmuls,
    free-dim softmax, PE transposes to assemble the output. Encoder tiles are
    DMA-prefetched at kernel start; PSUM evacuations are split between the
    DVE and ACT engines.

All matmuls use bf16 operands with f32 PSUM accumulation; gate arithmetic is
f32 (h is rounded to bf16 once per step). Host side does only sharding,
layout transposes, dtype casts, and the embedding gather.
"""

import numpy as np
import ml_dtypes

NB, S, H, E = 8, 128, 512, 512
G = 3 * H            # 1536
BT = NB * S          # 1024
NCORES = 8

_cache = {}


def _build():
    import concourse.bass as bass
    import concourse.bacc as bacc
    import concourse.mybir as mybir
    from concourse import tile
    from contextlib import ExitStack

    f32 = mybir.dt.float32
    bf16 = mybir.dt.bfloat16
    AF = mybir.ActivationFunctionType
    ALU = mybir.AluOpType
    PSUM = bass.MemorySpace.PSUM

    nc = bacc.Bacc(
        "TRN2",
        target_bir_lowering=False,
        debug=False,
        enable_asserts=False,
        num_devices=NCORES,
    )

    embedT_d = nc.dram_tensor("embedT", [E, BT], bf16, kind="ExternalInput")
    wih_d = nc.dram_tensor("W_ihT", [E, G], bf16, kind="ExternalInput")
    whh_d = nc.dram_tensor("W_hhT", [H, G], bf16, kind="ExternalInput")
    biascol_d = nc.dram_tensor("bias_col", [128, 12], f32, kind="ExternalInput")
    bhhn_d = nc.dram_tensor("bhh_n", [128, 4, NB], bf16, kind="ExternalInput")
    h0T_d = nc.dram_tensor("h0T", [H, NB], bf16, kind="ExternalInput")
    enc_d = nc.dram_tensor("enc", [NB, S, H], bf16, kind="ExternalInput")
    encT_d = nc.dram_tensor("encT", [NB, H, S], bf16, kind="ExternalInput")
    iden_d = nc.dram_tensor("iden", [128, 128], bf16, kind="ExternalInput")
    out_d = nc.dram_tensor("out", [NB, S, 2 * H], f32, kind="ExternalOutput")

    with tile.TileContext(nc) as tc, ExitStack() as ctx:
        cp = ctx.enter_context(tc.tile_pool(name="const", bufs=1))
        giRZ = cp.tile([128, 8, BT], bf16)           # r/z gate inputs (+biases)
        # Hall2[p, t, kc, b, 0] = n_t (unused), [.., 1] = h_t; column t holds
        # state entering step t. Written whole-column by the h-update scan.
        Hall2 = cp.tile([128, S + 1, 4, NB, 2], bf16)
        # D1[p, t, kc, b, 0] = r_t (sigmoid out), [.., 1] = gi_n (+b_ih).
        # scan2 d1 operand: odd slots pre-filled by Phase A.
        D1 = cp.tile([128, S, 4, NB, 2], f32)
        # d02: even = 0 (memset once), odd = gh_n(t) (PSUM evacuation).
        d02 = cp.tile([128, 4, NB, 2], f32)
        # d01: even = 0 (memset once), odd = (1-z)(t) (sigmoid out).
        d01 = cp.tile([128, 4, NB, 2], f32)
        whh = cp.tile([128, 4, G], bf16)
        wih = cp.tile([128, 4, G], bf16)
        embT = cp.tile([128, 4, BT], bf16)
        biascol = cp.tile([128, 12], f32)
        bhhn = cp.tile([128, 4, NB], bf16)
        iden = cp.tile([128, 128], bf16)
        encAll = cp.tile([128, NB, H], bf16)         # enc[b]: [s, h]
        encTAll = cp.tile([128, NB, 4, S], bf16)     # encT[b]: [p, hm, s]

        nc.sync.dma_start(whh[:], whh_d.ap().rearrange("(k p) g -> p k g", p=128))
        nc.sync.dma_start(wih[:], wih_d.ap().rearrange("(k p) g -> p k g", p=128))
        nc.sync.dma_start(embT[:], embedT_d.ap().rearrange("(k p) n -> p k n", p=128))
        nc.sync.dma_start(biascol[:], biascol_d.ap())
        nc.sync.dma_start(bhhn[:], bhhn_d.ap())
        h0t = cp.tile([128, 4, NB], bf16)
        nc.sync.dma_start(h0t[:], h0T_d.ap().rearrange("(k p) b -> p k b", p=128))
        nc.vector.tensor_copy(Hall2[:, 0, :, :, 1], h0t[:])
        nc.vector.memset(d02[:, :, :, 0], 0.0)
        nc.vector.memset(d01[:, :, :, 0], 0.0)
        nc.sync.dma_start(iden[:], iden_d.ap())
        for b in range(NB):
            nc.sync.dma_start(encAll[:, b, :], enc_d.ap()[b])
            nc.sync.dma_start(
                encTAll[:, b], encT_d.ap()[b].rearrange("(k p) s -> p k s", p=128)
            )

        # ---- Phase A: gi[:, m, :] = (W_ih chunk) @ embedT + biases ----
        with tc.tile_pool(name="psA", bufs=4, space=PSUM) as psA:
            for m in range(12):
                for n in range(2):
                    psa = psA.tile([128, 512], f32, tag="psa")
                    for k in range(4):
                        nc.tensor.matmul(
                            psa[:],
                            wih[:, k, 128 * m : 128 * (m + 1)],
                            embT[:, k, 512 * n : 512 * (n + 1)],
                            start=(k == 0),
                            stop=(k == 3),
                        )
                    if m < 8:
                        dst = giRZ[:, m, 512 * n : 512 * (n + 1)]
                        src = psa[:]
                    else:
                        # gi_n goes to the odd slots of D1 for steps
                        # 64n..64n+63; psa columns are (t*8+b)-ordered.
                        dst = D1[:, 64 * n : 64 * (n + 1), m - 8, :, 1]
                        src = psa[:].rearrange("p (t b) -> p t b", b=NB)
                    if (2 * m + n) % 2 == 0:
                        nc.vector.tensor_scalar_add(dst, src, biascol[:, m : m + 1])
                    else:
                        nc.scalar.activation(
                            dst, src, AF.Identity, bias=biascol[:, m : m + 1]
                        )

        # ---- Phase B: GRU recurrence, 128 serial steps ----
        # Weight m-index: m 0..3 = r gates, 4..7 = z gates (host-negated so
        # sigmoid gives 1-z directly), 8..11 = n gates; r/z/n accumulate in
        # THREE separate PSUM banks so each consumer waits only on its own
        # bank's PE writes. The elementwise chains are fused pairwise with
        # tensor_tensor_scan over interleaved operands:
        #   scan2: d0=[0|gh_n] d1=[r|gi_n]    -> odd out = r*gh_n + gi_n
        #   scan1: d0=[0|1-z]  d1=[n|z*h]     -> odd out = (1-z)*n + z*h
        # Critical path per step:
        #   h-MMs(r) -> sigmoid(r) -> scan2 -> tanh -> scan1 (= h update)
        with (
            tc.tile_pool(name="psB", bufs=2, space=PSUM) as psB,
            tc.tile_pool(name="gp", bufs=3) as gp,
        ):
            for t in range(S):
                h_src = Hall2[:, t, :, :, 1]         # [128, 4, NB] strided
                gsl = slice(8 * t, 8 * (t + 1))
                ps_r = psB.tile([128, 4, NB], f32, tag="ps_r", name="ps_r")
                ps_z = psB.tile([128, 4, NB], f32, tag="ps_z", name="ps_z")
                ps_n = psB.tile([128, 4, NB], f32, tag="ps_n", name="ps_n")
                # off-path: open the accumulations with identity matmuls
                # injecting gi_rz / b_hh_n. Only the first id-MM per bank
                # clears has_written (start=True wipes the WHOLE bank); the
                # others overwrite their stale slices, and the h-matmuls
                # accumulate on top.
                for m in range(4):
                    nc.tensor.matmul(
                        ps_r[:, m, :], iden[:], giRZ[:, m, gsl],
                        start=(m == 0), stop=False,
                    )
                for m in range(4):
                    nc.tensor.matmul(
                        ps_z[:, m, :], iden[:], giRZ[:, 4 + m, gsl],
                        start=(m == 0), stop=False,
                    )
                for j in range(4):
                    nc.tensor.matmul(
                        ps_n[:, j, :], iden[:], bhhn[:, j, :],
                        start=(j == 0), stop=False,
                    )
                # h-dependent matmuls: r tiles, then n, then z
                for m, dst in (
                    [(m, ps_r[:, m, :]) for m in range(4)]
                    + [(m, ps_n[:, m - 8, :]) for m in range(8, 12)]
                    + [(m, ps_z[:, m - 4, :]) for m in range(4, 8)]
                ):
                    for k in range(4):
                        nc.tensor.matmul(
                            dst,
                            whh[:, k, 128 * m : 128 * (m + 1)],
                            Hall2[:, t, k, :, 1],
                            start=False,
                            stop=(k == 3),
                        )
                srow = D1[:, t]                      # [128, 4, NB, 2]
                # sigmoid(r) straight into scan2's d1 even slots
                nc.scalar.activation(srow[:, :, :, 0], ps_r[:], AF.Sigmoid)
                # sigmoid(-z) = 1-z into scan1's d0 odd slots (off-path)
                nc.scalar.activation(d01[:, :, :, 1], ps_z[:], AF.Sigmoid)
                # gh_n evacuation into scan2's d0 odd slots (off-path)
                nc.vector.tensor_copy(d02[:, :, :, 1], ps_n[:])
                # scan2 odd out: tn3 = r*gh_n + gi_n
                s2 = gp.tile([128, 4, NB, 2], f32, tag="s2", name="s2")
                nc.vector.tensor_tensor_scan(
                    s2[:].rearrange("p a b c -> p (a b c)"),
                    d02[:].rearrange("p a b c -> p (a b c)"),
                    srow.rearrange("p a b c -> p (a b c)"),
                    0.0, ALU.mult, ALU.add,
                )
                d11 = gp.tile([128, 4, NB, 2], f32, tag="d11", name="d11")
                nc.scalar.activation(d11[:, :, :, 0], s2[:, :, :, 1], AF.Tanh)
                # off-path: z*h = h - (1-z)*h into scan1's d1 odd slots
                qq = gp.tile([128, 4, NB], f32, tag="qq", name="qq")
                nc.vector.tensor_mul(qq[:], d01[:, :, :, 1], h_src)
                nc.vector.tensor_sub(d11[:, :, :, 1], h_src, qq[:])
                # scan1 odd out: h_t = (1-z)*n + z*h  (whole column written)
                nc.vector.tensor_tensor_scan(
                    Hall2[:, t + 1].rearrange("p a b c -> p (a b c)"),
                    d01[:].rearrange("p a b c -> p (a b c)"),
                    d11[:].rearrange("p a b c -> p (a b c)"),
                    0.0, ALU.mult, ALU.add,
                )

        # ---- Phase C: attention + output assembly, per batch element ----
        with (
            tc.tile_pool(name="pc", bufs=2) as pc,
            tc.tile_pool(name="psC", bufs=2, space=PSUM) as psC,
            tc.tile_pool(name="psX", bufs=2, space=PSUM) as psX,
        ):
            for b in range(NB):
                ps_sc = psC.tile([128, 128], f32, tag="c128")
                for k in range(4):
                    nc.tensor.matmul(
                        ps_sc[:],
                        Hall2[:, 1 : S + 1, k, b, 1],
                        encTAll[:, b, k, :],
                        start=(k == 0),
                        stop=(k == 3),
                    )
                mxn = pc.tile([128, 1], f32, tag="mxn")
                nc.vector.tensor_reduce(
                    mxn[:], ps_sc[:], op=ALU.max,
                    axis=mybir.AxisListType.X, negate=True,
                )
                probs = pc.tile([128, 128], bf16, tag="probs")
                sm = pc.tile([128, 1], f32, tag="sm")
                nc.scalar.activation(
                    probs[:], ps_sc[:], AF.Exp, bias=mxn[:], accum_out=sm[:]
                )
                rs = pc.tile([128, 1], f32, tag="rs")
                nc.vector.reciprocal(rs[:], sm[:])
                ps_pt = psC.tile([128, 128], bf16, tag="c128b", bufs=4)
                nc.tensor.transpose(ps_pt[:], probs[:], iden[:])
                probsT = pc.tile([128, 128], bf16, tag="probsT")
                nc.scalar.activation(probsT[:], ps_pt[:], AF.Copy)
                ps_cx = psX.tile([128, 512], f32, tag="ctx")
                nc.tensor.matmul(
                    ps_cx[:], probsT[:], encAll[:, b, :], start=True, stop=True
                )
                y = pc.tile([128, 2 * H], f32, tag="y")
                for k in range(4):
                    ps_h = psC.tile([128, 128], bf16, tag="c128b", bufs=4)
                    nc.tensor.transpose(
                        ps_h[:], Hall2[:, 1 : S + 1, k, b, 1], iden[:]
                    )
                    if k % 2 == 0:
                        nc.vector.tensor_copy(y[:, 128 * k : 128 * (k + 1)], ps_h[:])
                    else:
                        nc.scalar.activation(
                            y[:, 128 * k : 128 * (k + 1)], ps_h[:], AF.Copy
                        )
                nc.vector.tensor_scalar_mul(y[:, H:], ps_cx[:], rs[:])
                nc.sync.dma_start(out_d.ap()[b], y[:])

    nc.compile()
    return nc


def _get_nc():
    if "nc" not in _cache:
        _cache["nc"] = _build()
    return _cache["nc"]


def prepare_in_maps(
    decoder_input,
    encoder_hidden,
    encoder_output,
    emb_table,
    W_ih,
    W_hh,
    b_ih,
    b_hh,
    epoch=0,
    **_unused,
):
    dec = np.asarray(decoder_input)
    enc_h = np.asarray(encoder_hidden, np.float32)[0]      # [64, 512]
    enc_o = np.asarray(encoder_output, np.float32)         # [64, 128, 512]
    emb = np.asarray(emb_table, np.float32)
    W_ih = np.asarray(W_ih, np.float32)
    W_hh = np.asarray(W_hh, np.float32)
    b_ih = np.asarray(b_ih, np.float32)
    b_hh = np.asarray(b_hh, np.float32)

    embed = emb[dec]                                       # [64, 128, 512] gather

    # Negate the z-gate rows (512:1024) of weights and biases so the device
    # computes -x_z in PSUM and a single sigmoid yields [r | 1-z] directly.
    W_ih = W_ih.copy(); W_ih[512:1024] *= -1.0
    W_hh = W_hh.copy(); W_hh[512:1024] *= -1.0
    b_ih = b_ih.copy(); b_ih[512:1024] *= -1.0
    b_hh = b_hh.copy(); b_hh[512:1024] *= -1.0

    WihT_bf = np.ascontiguousarray(W_ih.T).astype(ml_dtypes.bfloat16)
    WhhT_bf = np.ascontiguousarray(W_hh.T).astype(ml_dtypes.bfloat16)
    # bias_col[:, m] = b_ih chunk m, plus b_hh chunk for r/z gates (m < 8)
    bias_col = np.zeros((128, 12), np.float32)
    for m in range(12):
        bias_col[:, m] = b_ih[128 * m : 128 * (m + 1)]
        if m < 8:
            bias_col[:, m] += b_hh[128 * m : 128 * (m + 1)]
    # bhh_n[p, k, b] = b_hh[1024 + 128k + p]
    bhh_n = np.ascontiguousarray(
        np.repeat(b_hh[1024:].reshape(4, 128).T[:, :, None], NB, axis=2)
    ).astype(ml_dtypes.bfloat16)
    iden = np.eye(128, dtype=ml_dtypes.bfloat16)

    in_maps = []
    for c in range(NCORES):
        bs = slice(c * NB, (c + 1) * NB)
        embedT = np.ascontiguousarray(
            embed[bs].transpose(2, 1, 0).reshape(E, BT)
        ).astype(ml_dtypes.bfloat16)                       # [E, t*8+b]
        enc_c = enc_o[bs]
        in_maps.append(
            {
                "embedT": embedT,
                "W_ihT": WihT_bf,
                "W_hhT": WhhT_bf,
                "bias_col": bias_col,
                "bhh_n": bhh_n,
                "h0T": np.ascontiguousarray(enc_h[bs].T).astype(ml_dtypes.bfloat16),
                "enc": np.ascontiguousarray(enc_c).astype(ml_dtypes.bfloat16),
                "encT": np.ascontiguousarray(
                    enc_c.transpose(0, 2, 1)
                ).astype(ml_dtypes.bfloat16),
                "iden": iden,
            }
        )
    return in_maps


def assemble(results):
    out = np.empty((NCORES * NB, S, 2 * H), np.float32)
    for c in range(NCORES):
        out[c * NB : (c + 1) * NB] = results[c]["out"]
    return out


def kernel(**inputs):
    from concourse.bass_utils import run_bass_kernel_spmd

    in_maps = prepare_in_maps(**inputs)
    nc = _get_nc()
    _cache["in_maps"] = in_maps
    res = run_bass_kernel_spmd(nc, in_maps, core_ids=list(range(NCORES)))
    return assemble(res.results)


# revision 17
# speedup vs baseline: 2.3514x; 1.1045x over previous
"""Trainium2 Bass kernel for nn_AttentionDecoder (GRU decoder + dot attention).

Strategy (8 NeuronCores, data-parallel over batch, no collectives):
  - batch 64 -> 8 per core
  - Phase A (parallel): gi = W_ih @ embed^T for all timesteps (bf16 matmul);
    r/z gate halves (+b_ih+b_hh) stored bf16, n-gate half (+b_ih) stored f32.
  - Phase B (serial, 128 steps): GRU recurrence in transposed layout
    (gate-dim on partitions, batch on free dim). The critical cycle is
    minimized: gi_rz and b_hh_n are injected into the PSUM accumulation via
    identity matmuls (PE is idle anyway), so the on-path chain per step is
    just  MM -> sigmoid(r) -> mul -> add -> tanh -> mul -> add .  The z-gate
    products ((1-z) via sigmoid(-x), z*h) are computed off the critical path.
    h-matmuls are ordered r-tiles, n-tiles, z-tiles so sigmoid(r) can start
    as early as possible.
  - Phase C (parallel): attention per batch element via bf16 PE matmuls,
    free-dim softmax, PE transposes to assemble the output. Encoder tiles are
    DMA-prefetched at kernel start; PSUM evacuations are split between the
    DVE and ACT engines.

All matmuls use bf16 operands with f32 PSUM accumulation; gate arithmetic is
f32 (h is rounded to bf16 once per step). Host side does only sharding,
layout transposes, dtype casts, and the embedding gather.
"""

import numpy as np
import ml_dtypes

NB, S, H, E = 8, 128, 512, 512
G = 3 * H            # 1536
BT = NB * S          # 1024
NCORES = 8

_cache = {}


def _build():
    import concourse.bass as bass
    import concourse.bacc as bacc
    import concourse.mybir as mybir
    from concourse import tile
    from contextlib import ExitStack

    f32 = mybir.dt.float32
    bf16 = mybir.dt.bfloat16
    AF = mybir.ActivationFunctionType
    ALU = mybir.AluOpType
    PSUM = bass.MemorySpace.PSUM

    nc = bacc.Bacc(
        "TRN2",
        target_bir_lowering=False,
        debug=False,
        enable_asserts=False,
        num_devices=NCORES,
    )

    embedT_d = nc.dram_tensor("embedT", [E, BT], bf16, kind="ExternalInput")
    wih_d = nc.dram_tensor("W_ihT", [E, G], bf16, kind="ExternalInput")
    whh_d = nc.dram_tensor("W_hhT", [H, G], bf16, kind="ExternalInput")
    biascol_d = nc.dram_tensor("bias_col", [128, 12], f32, kind="ExternalInput")
    bhhn_d = nc.dram_tensor("bhh_n", [128, 4, NB], bf16, kind="ExternalInput")
    h0T_d = nc.dram_tensor("h0T", [H, NB], bf16, kind="ExternalInput")
    enc_d = nc.dram_tensor("enc", [NB, S, H], bf16, kind="ExternalInput")
    encT_d = nc.dram_tensor("encT", [NB, H, S], bf16, kind="ExternalInput")
    iden_d = nc.dram_tensor("iden", [128, 128], bf16, kind="ExternalInput")
    out_d = nc.dram_tensor("out", [NB, S, 2 * H], f32, kind="ExternalOutput")

    with tile.TileContext(nc) as tc, ExitStack() as ctx:
        cp = ctx.enter_context(tc.tile_pool(name="const", bufs=1))
        giRZ = cp.tile([128, 8, BT], bf16)           # r/z gate inputs (+biases)
        # Hall2[p, t, kc, b, 0] = n_t (unused), [.., 1] = h_t; column t holds
        # state entering step t. Written whole-column by the h-update scan.
        Hall2 = cp.tile([128, S + 1, 4, NB, 2], bf16)
        # D1[p, t, kc, b, 0] = r_t (sigmoid out), [.., 1] = gi_n (+b_ih).
        # scan2 d1 operand: odd slots pre-filled by Phase A.
        D1 = cp.tile([128, S, 4, NB, 2], f32)
        # d02: even = 0 (memset once), odd = gh_n(t) (PSUM evacuation).
        d02 = cp.tile([128, 4, NB, 2], f32)
        # d01: even = 0 (memset once), odd = (1-z)(t) (sigmoid out).
        d01 = cp.tile([128, 4, NB, 2], f32)
        whh = cp.tile([128, 4, G], bf16)
        wih = cp.tile([128, 4, G], bf16)
        embT = cp.tile([128, 4, BT], bf16)
        biascol = cp.tile([128, 12], f32)
        bhhn = cp.tile([128, 4, NB], bf16)
        iden = cp.tile([128, 128], bf16)
        encAll = cp.tile([128, NB, H], bf16)         # enc[b]: [s, h]
        encTAll = cp.tile([128, NB, 4, S], bf16)     # encT[b]: [p, hm, s]

        # DMA order tuned so the Phase A pre-roll and step 0 can start ASAP:
        # wih + first embT half + the small tiles + whh first, big tails last.
        wih_r = wih_d.ap().rearrange("(k p) g -> p k g", p=128)
        nc.sync.dma_start(wih[:, :, 0:768], wih_r[:, :, 0:768])
        emb_r = embedT_d.ap().rearrange("(k p) n -> p k n", p=128)
        nc.sync.dma_start(embT[:, :, 0:512], emb_r[:, :, 0:512])
        nc.sync.dma_start(biascol[:], biascol_d.ap())
        nc.sync.dma_start(bhhn[:], bhhn_d.ap())
        h0t = cp.tile([128, 4, NB], bf16)
        nc.sync.dma_start(h0t[:], h0T_d.ap().rearrange("(k p) b -> p k b", p=128))
        nc.sync.dma_start(iden[:], iden_d.ap())
        nc.sync.dma_start(wih[:, :, 768:G], wih_r[:, :, 768:G])
        whh_r = whh_d.ap().rearrange("(k p) g -> p k g", p=128)
        nc.sync.dma_start(whh[:, :, 0:768], whh_r[:, :, 0:768])
        nc.sync.dma_start(whh[:, :, 768:G], whh_r[:, :, 768:G])
        nc.sync.dma_start(embT[:, :, 512:BT], emb_r[:, :, 512:BT])
        nc.vector.tensor_copy(Hall2[:, 0, :, :, 1], h0t[:])
        nc.vector.memset(d02[:, :, :, 0], 0.0)
        nc.vector.memset(d01[:, :, :, 0], 0.0)
        negb = cp.tile([128, 1], f32)
        nc.vector.memset(negb[:], -60.0)
        for b in range(NB):
            nc.sync.dma_start(encAll[:, b, :], enc_d.ap()[b])
            nc.sync.dma_start(
                encTAll[:, b], encT_d.ap()[b].rearrange("(k p) s -> p k s", p=128)
            )

        # ---- Phase A: gi[:, m, c] = (W_ih chunk m) @ embedT[cols c] + bias.
        # Emitted in 128-column chunks (16 timesteps each); chunks 0-1 run
        # before step 0, the remaining 72 (m, c) lumps are interleaved one
        # per recurrence step into Phase B's idle engine windows (chunk c is
        # complete well before step 16c consumes it).
        def emit_A(psA, m, c):
            psa = psA.tile([128, 128], f32, tag="psa", name="psa")
            for k in range(4):
                nc.tensor.matmul(
                    psa[:],
                    wih[:, k, 128 * m : 128 * (m + 1)],
                    embT[:, k, 128 * c : 128 * (c + 1)],
                    start=(k == 0),
                    stop=(k == 3),
                )
            if m < 8:
                dst = giRZ[:, m, 128 * c : 128 * (c + 1)]
                src = psa[:]
            else:
                # gi_n goes to the odd slots of D1 for steps 16c..16c+15;
                # psa columns are (t*8+b)-ordered.
                dst = D1[:, 16 * c : 16 * (c + 1), m - 8, :, 1]
                src = psa[:].rearrange("p (t b) -> p t b", b=NB)
            # gi_n evacs go to ACT so scan2's dependency on them folds into
            # its existing ACT wait; gi_rz evacs go to DVE for balance.
            if m < 8:
                nc.vector.tensor_scalar_add(dst, src, biascol[:, m : m + 1])
            else:
                nc.scalar.activation(
                    dst, src, AF.Identity, bias=biascol[:, m : m + 1]
                )

        rest_lumps = [(m, c) for c in range(2, 8) for m in range(12)]

        # ---- Phase B: GRU recurrence, 128 serial steps ----
        # Weight m-index: m 0..3 = r gates, 4..7 = z gates (host-negated so
        # sigmoid gives 1-z directly), 8..11 = n gates; r/z/n accumulate in
        # THREE separate PSUM banks so each consumer waits only on its own
        # bank's PE writes. The elementwise chains are fused pairwise with
        # tensor_tensor_scan over interleaved operands:
        #   scan2: d0=[0|gh_n] d1=[r|gi_n]    -> odd out = r*gh_n + gi_n
        #   scan1: d0=[0|1-z]  d1=[n|z*h]     -> odd out = (1-z)*n + z*h
        # Critical path per step:
        #   h-MMs(r) -> sigmoid(r) -> scan2 -> tanh -> scan1 (= h update)
        with (
            tc.tile_pool(name="psA", bufs=2, space=PSUM) as psA,
            tc.tile_pool(name="psB", bufs=2, space=PSUM) as psB,
            tc.tile_pool(name="gp", bufs=3) as gp,
        ):
            for c in range(2):
                for m in range(12):
                    emit_A(psA, m, c)
            for t in range(S):
                h_src = Hall2[:, t, :, :, 1]         # [128, 4, NB] strided
                gsl = slice(8 * t, 8 * (t + 1))
                ps_r = psB.tile([128, 4, NB], f32, tag="ps_r", name="ps_r")
                ps_z = psB.tile([128, 4, NB], f32, tag="ps_z", name="ps_z")
                ps_n = psB.tile([128, 4, NB], f32, tag="ps_n", name="ps_n")
                # off-path: open the accumulations with identity matmuls
                # injecting gi_rz / b_hh_n. Only the first id-MM per bank
                # clears has_written (start=True wipes the WHOLE bank); the
                # others overwrite their stale slices, and the h-matmuls
                # accumulate on top.
                for m in range(4):
                    nc.tensor.matmul(
                        ps_r[:, m, :], iden[:], giRZ[:, m, gsl],
                        start=(m == 0), stop=False,
                    )
                for m in range(4):
                    nc.tensor.matmul(
                        ps_z[:, m, :], iden[:], giRZ[:, 4 + m, gsl],
                        start=(m == 0), stop=False,
                    )
                for j in range(4):
                    nc.tensor.matmul(
                        ps_n[:, j, :], iden[:], bhhn[:, j, :],
                        start=(j == 0), stop=False,
                    )
                # h-dependent matmuls: r tiles, then n, then z
                for m, dst in (
                    [(m, ps_r[:, m, :]) for m in range(4)]
                    + [(m, ps_n[:, m - 8, :]) for m in range(8, 12)]
                    + [(m, ps_z[:, m - 4, :]) for m in range(4, 8)]
                ):
                    for k in range(4):
                        nc.tensor.matmul(
                            dst,
                            whh[:, k, 128 * m : 128 * (m + 1)],
                            Hall2[:, t, k, :, 1],
                            start=False,
                            stop=(k == 3),
                        )
                srow = D1[:, t]                      # [128, 4, NB, 2]
                # sigmoid(r) straight into scan2's d1 even slots
                nc.scalar.activation(srow[:, :, :, 0], ps_r[:], AF.Sigmoid)
                # sigmoid(-z) = 1-z into scan1's d0 odd slots (off-path)
                nc.scalar.activation(d01[:, :, :, 1], ps_z[:], AF.Sigmoid)
                # gh_n evacuation into scan2's d0 odd slots (off-path)
                nc.vector.tensor_copy(d02[:, :, :, 1], ps_n[:])
                # scan2 odd out: tn3 = r*gh_n + gi_n
                s2 = gp.tile([128, 4, NB, 2], f32, tag="s2", name="s2")
                nc.vector.tensor_tensor_scan(
                    s2[:].rearrange("p a b c -> p (a b c)"),
                    d02[:].rearrange("p a b c -> p (a b c)"),
                    srow.rearrange("p a b c -> p (a b c)"),
                    0.0, ALU.mult, ALU.add,
                )
                d11 = gp.tile([128, 4, NB, 2], f32, tag="d11", name="d11")
                nc.scalar.activation(d11[:, :, :, 0], s2[:, :, :, 1], AF.Tanh)
                # off-path: z*h = h - (1-z)*h into scan1's d1 odd slots
                qq = gp.tile([128, 4, NB], f32, tag="qq", name="qq")
                nc.vector.tensor_mul(qq[:], d01[:, :, :, 1], h_src)
                nc.vector.tensor_sub(d11[:, :, :, 1], h_src, qq[:])
                # scan1 odd out: h_t = (1-z)*n + z*h  (whole column written)
                nc.vector.tensor_tensor_scan(
                    Hall2[:, t + 1].rearrange("p a b c -> p (a b c)"),
                    d01[:].rearrange("p a b c -> p (a b c)"),
                    d11[:].rearrange("p a b c -> p (a b c)"),
                    0.0, ALU.mult, ALU.add,
                )
                # one interleaved Phase A lump per step, hidden in idle time
                if t < len(rest_lumps):
                    emit_A(psA, *rest_lumps[t])

        # ---- Phase C: attention + output assembly, per batch element ----
        # exp uses a constant -60 bias instead of a max-reduce: softmax is
        # shift-invariant and scores stay well inside f32 exp range (the max
        # of 128 zero-mean dots is nonnegative, so the sum never underflows).
        with (
            tc.tile_pool(name="pc", bufs=3) as pc,
            tc.tile_pool(name="psC", bufs=2, space=PSUM) as psC,
            tc.tile_pool(name="psX", bufs=2, space=PSUM) as psX,
        ):
            for b in range(NB):
                ps_sc = psC.tile([128, 128], f32, tag="c128")
                for k in range(4):
                    nc.tensor.matmul(
                        ps_sc[:],
                        Hall2[:, 1 : S + 1, k, b, 1],
                        encTAll[:, b, k, :],
                        start=(k == 0),
                        stop=(k == 3),
                    )
                probs = pc.tile([128, 128], bf16, tag="probs")
                sm = pc.tile([128, 1], f32, tag="sm")
                nc.scalar.activation(
                    probs[:], ps_sc[:], AF.Exp, bias=negb[:], accum_out=sm[:]
                )
                rs = pc.tile([128, 1], f32, tag="rs")
                nc.vector.reciprocal(rs[:], sm[:])
                ps_pt = psC.tile([128, 128], bf16, tag="c128b", bufs=2)
                nc.tensor.transpose(ps_pt[:], probs[:], iden[:])
                probsT = pc.tile([128, 128], bf16, tag="probsT")
                nc.scalar.activation(probsT[:], ps_pt[:], AF.Copy)
                ps_cx = psX.tile([128, 512], f32, tag="ctx")
                nc.tensor.matmul(
                    ps_cx[:], probsT[:], encAll[:, b, :], start=True, stop=True
                )
                y = pc.tile([128, 2 * H], f32, tag="y")
                for half in range(2):
                    ps_h2 = psC.tile([128, 256], bf16, tag="c256", bufs=2)
                    for j in range(2):
                        k = 2 * half + j
                        nc.tensor.transpose(
                            ps_h2[:, 128 * j : 128 * (j + 1)],
                            Hall2[:, 1 : S + 1, k, b, 1],
                            iden[:],
                        )
                    if half == 0:
                        nc.vector.tensor_copy(y[:, 0:256], ps_h2[:])
                    else:
                        nc.scalar.activation(y[:, 256:512], ps_h2[:], AF.Copy)
                nc.vector.tensor_scalar_mul(y[:, H:], ps_cx[:], rs[:])
                nc.sync.dma_start(out_d.ap()[b], y[:])

    nc.compile()
    return nc


def _get_nc():
    if "nc" not in _cache:
        _cache["nc"] = _build()
    return _cache["nc"]


def prepare_in_maps(
    decoder_input,
    encoder_hidden,
    encoder_output,
    emb_table,
    W_ih,
    W_hh,
    b_ih,
    b_hh,
    epoch=0,
    **_unused,
):
    dec = np.asarray(decoder_input)
    enc_h = np.asarray(encoder_hidden, np.float32)[0]      # [64, 512]
    enc_o = np.asarray(encoder_output, np.float32)         # [64, 128, 512]
    emb = np.asarray(emb_table, np.float32)
    W_ih = np.asarray(W_ih, np.float32)
    W_hh = np.asarray(W_hh, np.float32)
    b_ih = np.asarray(b_ih, np.float32)
    b_hh = np.asarray(b_hh, np.float32)

    embed = emb[dec]                                       # [64, 128, 512] gather

    # Negate the z-gate rows (512:1024) of weights and biases so the device
    # computes -x_z in PSUM and a single sigmoid yields [r | 1-z] directly.
    W_ih = W_ih.copy(); W_ih[512:1024] *= -1.0
    W_hh = W_hh.copy(); W_hh[512:1024] *= -1.0
    b_ih = b_ih.copy(); b_ih[512:1024] *= -1.0
    b_hh = b_hh.copy(); b_hh[512:1024] *= -1.0

    WihT_bf = np.ascontiguousarray(W_ih.T).astype(ml_dtypes.bfloat16)
    WhhT_bf = np.ascontiguousarray(W_hh.T).astype(ml_dtypes.bfloat16)
    # bias_col[:, m] = b_ih chunk m, plus b_hh chunk for r/z gates (m < 8)
    bias_col = np.zeros((128, 12), np.float32)
    for m in range(12):
        bias_col[:, m] = b_ih[128 * m : 128 * (m + 1)]
        if m < 8:
            bias_col[:, m] += b_hh[128 * m : 128 * (m + 1)]
    # bhh_n[p, k, b] = b_hh[1024 + 128k + p]
    bhh_n = np.ascontiguousarray(
        np.repeat(b_hh[1024:].reshape(4, 128).T[:, :, None], NB, axis=2)
    ).astype(ml_dtypes.bfloat16)
    iden = np.eye(128, dtype=ml_dtypes.bfloat16)

    in_maps = []
    for c in range(NCORES):
        bs = slice(c * NB, (c + 1) * NB)
        embedT = np.ascontiguousarray(
            embed[bs].transpose(2, 1, 0).reshape(E, BT)
        ).astype(ml_dtypes.bfloat16)                       # [E, t*8+b]
        enc_c = enc_o[bs]
        in_maps.append(
            {
                "embedT": embedT,
                "W_ihT": WihT_bf,
                "W_hhT": WhhT_bf,
                "bias_col": bias_col,
                "bhh_n": bhh_n,
                "h0T": np.ascontiguousarray(enc_h[bs].T).astype(ml_dtypes.bfloat16),
                "enc": np.ascontiguousarray(enc_c).astype(ml_dtypes.bfloat16),
                "encT": np.ascontiguousarray(
                    enc_c.transpose(0, 2, 1)
                ).astype(ml_dtypes.bfloat16),
                "iden": iden,
            }
        )
    return in_maps


def assemble(results):
    out = np.empty((NCORES * NB, S, 2 * H), np.float32)
    for c in range(NCORES):
        out[c * NB : (c + 1) * NB] = results[c]["out"]
    return out


def kernel(**inputs):
    from concourse.bass_utils import run_bass_kernel_spmd

    in_maps = prepare_in_maps(**inputs)
    nc = _get_nc()
    _cache["in_maps"] = in_maps
    res = run_bass_kernel_spmd(nc, in_maps, core_ids=list(range(NCORES)))
    return assemble(res.results)
